# revision 13
# baseline (speedup 1.0000x reference)
"""AttnBlock (GroupNorm + single-head self-attention over 4096 tokens) on 8 trn2 cores.

Sharding: core i handles batch b=i//4, query tokens s=i%4 -> [s*1024, (s+1)*1024).
Each core recomputes GroupNorm + K/V projections for its batch (no collectives),
computes Q/attention/out-proj for its 1024-query slice, returns [512, 1024] fp32.

Layout: channels on SBUF partitions (4 ptiles of 128). Scores kept transposed
[m_part, nq_free] so softmax normalization uses a ones-matmul for the denominator
and exp never needs a cross-partition reduction (inputs are unit-variance; no
max-subtraction needed, verified |score*scale| < ~7 << 88).
"""

import sys

sys.path.insert(0, "/opt/trn_rl_repo")

import numpy as np
import ml_dtypes

B, C, H, W = 2, 512, 64, 64
N = H * W            # 4096 key/value tokens
NQ = N // 4          # 1024 query tokens per core
PT = C // 128        # 4 channel partition-tiles
NCHUNK = N // 512    # 8 key chunks of 512
NMT = N // 128       # 32 key m-tiles of 128
QCH = NQ // 512      # 2 query chunks of 512
GS = 16              # channels per group
EPS = 1e-6
SCALE = float(C) ** -0.5

_CACHE = {}


def _build():
    import concourse.bass as bass
    import concourse.bacc as bacc
    import concourse.tile as tile
    from concourse import mybir
    from contextlib import ExitStack

    f32 = mybir.dt.float32
    bf16 = mybir.dt.bfloat16
    Alu = mybir.AluOpType
    Act = mybir.ActivationFunctionType

    nc = bacc.Bacc("TRN2")

    # ---- I/O ----
    xb = nc.dram_tensor("xb", [C, N], f32, kind="ExternalInput")
    xq = nc.dram_tensor("xq", [C, NQ], f32, kind="ExternalInput")
    wqT = nc.dram_tensor("wqT", [C, C], bf16, kind="ExternalInput")
    wkT = nc.dram_tensor("wkT", [C, C], bf16, kind="ExternalInput")
    wvT = nc.dram_tensor("wvT", [C, C], bf16, kind="ExternalInput")
    woT = nc.dram_tensor("woT", [C, C], bf16, kind="ExternalInput")
    bq = nc.dram_tensor("bq", [C, 1], f32, kind="ExternalInput")
    bk = nc.dram_tensor("bk", [C, 1], f32, kind="ExternalInput")
    bv = nc.dram_tensor("bv", [C, 1], f32, kind="ExternalInput")
    bo = nc.dram_tensor("bo", [C, 1], f32, kind="ExternalInput")
    gnsc = nc.dram_tensor("gnsc", [C, 1], f32, kind="ExternalInput")
    gnbi = nc.dram_tensor("gnbi", [C, 1], f32, kind="ExternalInput")
    ind = nc.dram_tensor("ind", [128, 8], f32, kind="ExternalInput")    # 1/16 group indicator
    expand = nc.dram_tensor("expand", [8, 128], f32, kind="ExternalInput")  # group -> channel
    o = nc.dram_tensor("o", [C, NQ], f32, kind="ExternalOutput")

    with tile.TileContext(nc) as tc, ExitStack() as outer:
        # ---- pools live for the whole kernel ----
        k_pool = outer.enter_context(tc.tile_pool(name="k", bufs=1))
        vt_pool = outer.enter_context(tc.tile_pool(name="vt", bufs=1))
        q_pool = outer.enter_context(tc.tile_pool(name="q", bufs=1))
        xq_pool = outer.enter_context(tc.tile_pool(name="xq", bufs=1))
        wo_pool = outer.enter_context(tc.tile_pool(name="wo", bufs=1))
        const_pool = outer.enter_context(tc.tile_pool(name="const", bufs=1))

        ones_t = const_pool.tile([128, 1], bf16, tag="ones")
        nc.vector.memset(ones_t, 1.0)
        zero128 = const_pool.tile([128, 1], f32, tag="zero128")
        nc.vector.memset(zero128, 0.0)
        eps8 = const_pool.tile([8, 1], f32, tag="eps8")
        nc.vector.memset(eps8, EPS)
        ones_row = const_pool.tile([1, 128], f32, tag="ones_row")
        nc.vector.memset(ones_row, 1.0)
        bq_t, bk_t, bv_t, bo_t = [], [], [], []
        for ci in range(PT):
            for bi, (dst, src) in enumerate(((bq_t, bq), (bk_t, bk), (bv_t, bv), (bo_t, bo))):
                t = const_pool.tile([128, 1], f32, tag=f"bias{bi}_{ci}")
                nc.sync.dma_start(out=t, in_=src[ci * 128:(ci + 1) * 128, :])
                dst.append(t)
        wo_t = []
        for ci in range(PT):
            t = wo_pool.tile([128, C], bf16, name=f"wo{ci}", tag=f"wo{ci}")
            nc.sync.dma_start(out=t, in_=woT[ci * 128:(ci + 1) * 128, :])
            wo_t.append(t)
        xq_t = []
        for ci in range(PT):
            t = xq_pool.tile([128, NQ], f32, name=f"xqt{ci}", tag=f"xqt{ci}")
            nc.sync.dma_start(out=t, in_=xq[ci * 128:(ci + 1) * 128, :])
            xq_t.append(t)

        bv_v, bo_v = [], []
        for ci in range(PT):
            t = const_pool.tile([128, 1], f32, tag=f"bvv{ci}")
            nc.vector.tensor_copy(t, bv_t[ci])
            bv_v.append(t)
            t = const_pool.tile([128, 1], f32, tag=f"bov{ci}")
            nc.vector.tensor_copy(t, bo_t[ci])
            bo_v.append(t)
        xqv_t = []
        for ci in range(PT):
            t = xq_pool.tile([128, NQ], f32, name=f"xqv{ci}", tag=f"xqv{ci}")
            nc.vector.tensor_copy(t, xq_t[ci])
            xqv_t.append(t)

        kt = [k_pool.tile([128, N], bf16, name=f"kt{i}", tag=f"kt{i}") for i in range(PT)]
        vt = [vt_pool.tile([128, C], bf16, name=f"vt{i}", tag=f"vt{i}") for i in range(NMT)]
        qt = [q_pool.tile([128, NQ], bf16, name=f"qt{i}", tag=f"qt{i}") for i in range(PT)]

        # ================= Phase A: GroupNorm + projections =================
        with ExitStack() as ph1:
            xb_pool = ph1.enter_context(tc.tile_pool(name="xb", bufs=2))
            hn_pool = ph1.enter_context(tc.tile_pool(name="hn", bufs=PT))
            hnq_pool = ph1.enter_context(tc.tile_pool(name="hnq", bufs=PT))
            w_pool = ph1.enter_context(tc.tile_pool(name="w", bufs=1))
            st_pool = ph1.enter_context(tc.tile_pool(name="st", bufs=2))
            sm_pool = ph1.enter_context(tc.tile_pool(name="sm", bufs=2))
            gc_pool = ph1.enter_context(tc.tile_pool(name="gc", bufs=1))
            psA = ph1.enter_context(tc.tile_pool(name="psA", bufs=2, space="PSUM"))
            psS = ph1.enter_context(tc.tile_pool(name="psS", bufs=2, space="PSUM"))

            ind_dma = gc_pool.tile([128, 8], f32, tag="ind_dma")
            nc.sync.dma_start(out=ind_dma, in_=ind[:, :])
            ind_t = gc_pool.tile([128, 8], f32, tag="ind")
            nc.vector.tensor_copy(ind_t, ind_dma)
            exp_dma = gc_pool.tile([8, 128], f32, tag="expand_dma")
            nc.sync.dma_start(out=exp_dma, in_=expand[:, :])
            exp_t = gc_pool.tile([8, 128], f32, tag="expand")
            nc.vector.tensor_copy(exp_t, exp_dma)
            gnsc_t, gnbi_t = [], []
            for ci in range(PT):
                td = gc_pool.tile([128, 1], f32, tag=f"gnscd{ci}")
                nc.sync.dma_start(out=td, in_=gnsc[ci * 128:(ci + 1) * 128, :])
                t = gc_pool.tile([128, 1], f32, tag=f"gnsc{ci}")
                nc.vector.tensor_copy(t, td)
                gnsc_t.append(t)
                td = gc_pool.tile([128, 1], f32, tag=f"gnbid{ci}")
                nc.sync.dma_start(out=td, in_=gnbi[ci * 128:(ci + 1) * 128, :])
                t = gc_pool.tile([128, 1], f32, tag=f"gnbi{ci}")
                nc.vector.tensor_copy(t, td)
                gnbi_t.append(t)

            wq_t, wk_t, wv_t = [], [], []
            for wi, (dst, src) in enumerate(((wq_t, wqT), (wk_t, wkT), (wv_t, wvT))):
                for ci in range(PT):
                    t = w_pool.tile([128, C], bf16, tag=f"w{wi}_{ci}")
                    nc.sync.dma_start(out=t, in_=src[ci * 128:(ci + 1) * 128, :])
                    dst.append(t)

            hn_t, hnq_t = [], []
            for ci in range(PT):
                xbt = xb_pool.tile([128, N], f32)
                nc.sync.dma_start(out=xbt, in_=xb[ci * 128:(ci + 1) * 128, :])

                stats = st_pool.tile([128, NCHUNK, 6], f32)
                for j in range(NCHUNK):
                    nc.vector.bn_stats(out=stats[:, j, :], in_=xbt[:, j * 512:(j + 1) * 512])
                mv = sm_pool.tile([128, 2], f32, tag="mv")
                nc.vector.bn_aggr(out=mv, in_=stats)
                m2 = sm_pool.tile([128, 1], f32, tag="m2")
                nc.vector.tensor_mul(m2, mv[:, 0:1], mv[:, 0:1])

                # group averages of (mean, var, mean^2): ind holds 1/16
                ps3 = psA.tile([8, 3], f32, tag="ps3")
                nc.tensor.matmul(ps3[:, 0:1], ind_t, mv[:, 0:1], start=True, stop=True)
                nc.tensor.matmul(ps3[:, 1:2], ind_t, mv[:, 1:2], start=True, stop=True)
                nc.tensor.matmul(ps3[:, 2:3], ind_t, m2, start=True, stop=True)

                s3 = sm_pool.tile([8, 3], f32, tag="s3")
                nc.vector.tensor_copy(s3, ps3)
                mean8 = s3[:, 0:1]
                ex2 = sm_pool.tile([8, 1], f32, tag="ex2")
                nc.vector.tensor_tensor(ex2, s3[:, 1:2], s3[:, 2:3], Alu.add)
                m28 = sm_pool.tile([8, 1], f32, tag="m28")
                nc.vector.tensor_mul(m28, mean8, mean8)
                var8 = sm_pool.tile([8, 1], f32, tag="var8")
                nc.vector.tensor_tensor(var8, ex2, m28, Alu.subtract)
                sd8 = sm_pool.tile([8, 1], f32, tag="sd8")
                nc.scalar.activation(out=sd8, in_=var8, func=Act.Sqrt, bias=eps8)
                r8 = sm_pool.tile([8, 1], f32, tag="r8")
                nc.vector.reciprocal(r8, sd8)

                # broadcast group stats back to 128 channels
                psmr = psA.tile([128, 2], f32, tag="psmr")
                nc.tensor.matmul(psmr[:, 0:1], exp_t, mean8, start=True, stop=True)
                nc.tensor.matmul(psmr[:, 1:2], exp_t, r8, start=True, stop=True)

                A_t = sm_pool.tile([128, 1], f32, tag="A")
                nc.vector.tensor_mul(A_t, psmr[:, 1:2], gnsc_t[ci])
                tB = sm_pool.tile([128, 1], f32, tag="tB")
                nc.vector.tensor_mul(tB, psmr[:, 0:1], A_t)
                B_t = sm_pool.tile([128, 1], f32, tag="B")
                nc.vector.tensor_tensor(B_t, gnbi_t[ci], tB, Alu.subtract)

                hnt = hn_pool.tile([128, N], bf16)
                nc.vector.tensor_scalar(out=hnt, in0=xbt, scalar1=A_t, scalar2=B_t,
                                        op0=Alu.mult, op1=Alu.add)
                hn_t.append(hnt)
                hq = hnq_pool.tile([128, NQ], bf16)
                nc.vector.tensor_scalar(out=hq, in0=xq_t[ci], scalar1=A_t, scalar2=B_t,
                                        op0=Alu.mult, op1=Alu.add)
                hnq_t.append(hq)

            # Q projection: q[d, n] = sum_c wqT[c, d] hnq[c, n] + bq[d]
            for di in range(PT):
                for ch in range(QCH):
                    ps = psS.tile([128, 512], f32, tag="ps")
                    for ci in range(PT):
                        nc.tensor.matmul(ps, wq_t[ci][:, di * 128:(di + 1) * 128],
                                         hnq_t[ci][:, ch * 512:(ch + 1) * 512],
                                         start=(ci == 0), stop=(ci == PT - 1))
                    nc.scalar.activation(out=qt[di][:, ch * 512:(ch + 1) * 512], in_=ps,
                                         func=Act.Identity, bias=bq_t[di])

            # K (by 512-col chunks) and Vt (by 128-row m-tiles), interleaved in m order
            for ch8 in range(NCHUNK):
                for di in range(PT):
                    ps = psS.tile([128, 512], f32, tag="ps")
                    for ci in range(PT):
                        nc.tensor.matmul(ps, wk_t[ci][:, di * 128:(di + 1) * 128],
                                         hn_t[ci][:, ch8 * 512:(ch8 + 1) * 512],
                                         start=(ci == 0), stop=(ci == PT - 1))
                    nc.scalar.activation(out=kt[di][:, ch8 * 512:(ch8 + 1) * 512], in_=ps,
                                         func=Act.Identity, bias=bk_t[di])
                for mi in range(ch8 * 4, (ch8 + 1) * 4):
                    ps = psS.tile([128, 512], f32, tag="ps")
                    for ci in range(PT):
                        nc.tensor.matmul(ps, hn_t[ci][:, mi * 128:(mi + 1) * 128],
                                         wv_t[ci],
                                         start=(ci == 0), stop=(ci == PT - 1))
                    nc.scalar.activation(out=vt[mi], in_=ps, func=Act.Copy)

        # ================= Phase B: attention + output projection =================
        with ExitStack() as ph2:
            ps_sc = ph2.enter_context(tc.tile_pool(name="ps_sc", bufs=2, space="PSUM"))
            ps_at = ph2.enter_context(tc.tile_pool(name="ps_at", bufs=1, space="PSUM"))
            ps_dn = ph2.enter_context(tc.tile_pool(name="ps_dn", bufs=2, space="PSUM"))
            p_pool = ph2.enter_context(tc.tile_pool(name="p", bufs=6))
            r_pool = ph2.enter_context(tc.tile_pool(name="r", bufs=2))
            R_pool = ph2.enter_context(tc.tile_pool(name="R", bufs=2))
            h_pool = ph2.enter_context(tc.tile_pool(name="h", bufs=2))
            o_pool = ph2.enter_context(tc.tile_pool(name="o", bufs=4))

            for ch in range(QCH):
                at = [ps_at.tile([128, 512], f32, name=f"at{di}", tag=f"at{di}") for di in range(PT)]
                dn = ps_dn.tile([1, 512], f32, tag="dn")
                for mi in range(NMT):
                    ps = ps_sc.tile([128, 512], f32, tag="sc")
                    for di in range(PT):
                        nc.tensor.matmul(ps, kt[di][:, mi * 128:(mi + 1) * 128],
                                         qt[di][:, ch * 512:(ch + 1) * 512],
                                         start=(di == 0), stop=(di == PT - 1))
                    pt = p_pool.tile([128, 512], bf16, tag="pt")
                    nc.scalar.activation(out=pt, in_=ps, func=Act.Exp, bias=zero128, scale=SCALE)
                    nc.tensor.matmul(dn, ones_t, pt, start=(mi == 0), stop=(mi == NMT - 1))
                    for di in range(PT):
                        nc.tensor.matmul(at[di], vt[mi][:, di * 128:(di + 1) * 128], pt,
                                         start=(mi == 0), stop=(mi == NMT - 1))

                r = r_pool.tile([1, 512], f32, tag="r")
                nc.vector.reciprocal(r, dn)
                Rp = ps_sc.tile([128, 512], f32, tag="sc")
                nc.tensor.matmul(Rp, ones_row, r, start=True, stop=True)
                Rt = R_pool.tile([128, 512], f32, tag="R")
                nc.vector.tensor_copy(Rt, Rp)

                ht = []
                for di in range(PT):
                    t = h_pool.tile([128, 512], bf16, tag=f"h{di}")
                    nc.vector.tensor_tensor(t, at[di], Rt, Alu.mult)
                    nc.vector.tensor_scalar_add(t, t, bv_v[di])
                    ht.append(t)

                for di in range(PT):
                    pso = ps_sc.tile([128, 512], f32, tag="sc")
                    for ci in range(PT):
                        nc.tensor.matmul(pso, wo_t[ci][:, di * 128:(di + 1) * 128], ht[ci],
                                         start=(ci == 0), stop=(ci == PT - 1))
                    ot = o_pool.tile([128, 512], f32, tag="ot")
                    nc.vector.scalar_tensor_tensor(
                        out=ot, in0=pso, scalar=bo_v[di],
                        in1=xqv_t[di][:, ch * 512:(ch + 1) * 512],
                        op0=Alu.add, op1=Alu.add)
                    nc.sync.dma_start(
                        out=o[di * 128:(di + 1) * 128, ch * 512:(ch + 1) * 512], in_=ot)

    nc.finalize()
    return nc


def _fix_multiwait_json(raw: bytes) -> bytes:
    """Walrus codegen allows at most ~2 sync commands per ISA instruction.
    Hoist every instruction's multi-wait set onto a Drain inserted just before
    it on the same engine (Drain is the one opcode walrus lowers with an
    arbitrary wait list - the standard kernel-tail drain relies on that)."""
    import json as _json

    m = _json.loads(raw)
    n = [0]

    def fix_block(b):
        out = []
        for inst in b.get("instructions", []):
            si = inst.get("sync_info")
            if si and inst.get("opcode") != "Drain" and len(si.get("on_wait") or []) >= 2:
                n[0] += 1
                out.append({
                    "engine": inst["engine"], "ins": [], "outs": [],
                    "name": f"waitfix_{n[0]}", "opcode": "Drain",
                    "is_reset_sema": False, "debug": inst.get("debug"),
                    "sync_info": {"on_update": [], "on_wait": si["on_wait"]},
                })
                si["on_wait"] = []
            out.append(inst)
        b["instructions"] = out
        for sb in b.get("blocks", []) or []:
            fix_block(sb)

    for fn in m["functions"]:
        for b in fn["blocks"]:
            fix_block(b)
    return _json.dumps(m).encode()


def _prep_inputs(x, gn_scale, gn_bias, wq, bq, wk, bk, wv, bv, wo, bo):
    bf = ml_dtypes.bfloat16
    xf = np.ascontiguousarray(np.asarray(x, np.float32).reshape(B, C, N))
    base = {
        "wqT": np.ascontiguousarray(np.asarray(wq, np.float32).T.astype(bf)),
        "wkT": np.ascontiguousarray(np.asarray(wk, np.float32).T.astype(bf)),
        "wvT": np.ascontiguousarray(np.asarray(wv, np.float32).T.astype(bf)),
        "woT": np.ascontiguousarray(np.asarray(wo, np.float32).T.astype(bf)),
        "bq": np.asarray(bq, np.float32).reshape(C, 1).copy(),
        "bk": np.asarray(bk, np.float32).reshape(C, 1).copy(),
        "bv": np.asarray(bv, np.float32).reshape(C, 1).copy(),
        "bo": np.asarray(bo, np.float32).reshape(C, 1).copy(),
        "gnsc": np.asarray(gn_scale, np.float32).reshape(C, 1).copy(),
        "gnbi": np.asarray(gn_bias, np.float32).reshape(C, 1).copy(),
        "ind": np.ascontiguousarray(
            (np.arange(128)[:, None] // GS == np.arange(8)[None, :]) / GS
        ).astype(np.float32),
        "expand": np.ascontiguousarray(
            (np.arange(128)[None, :] // GS == np.arange(8)[:, None])
        ).astype(np.float32),
    }
    in_maps = []
    for core in range(8):
        b, s = core // 4, core % 4
        m = dict(base)
        m["xb"] = np.ascontiguousarray(xf[b])
        m["xq"] = np.ascontiguousarray(xf[b][:, s * NQ:(s + 1) * NQ])
        in_maps.append(m)
    return in_maps


def kernel(x, gn_scale, gn_bias, wq, bq, wk, bk, wv, bv, wo, bo):
    import os
    from concourse.bass_utils import run_bass_kernel_spmd

    if "nc" not in _CACHE:
        _CACHE["nc"] = _build()
    nc = _CACHE["nc"]

    in_maps = _prep_inputs(x, gn_scale, gn_bias, wq, bq, wk, bk, wv, bv, wo, bo)
    trace = bool(int(os.environ.get("KERNEL_TRACE", "0")))
    res = run_bass_kernel_spmd(nc, in_maps, core_ids=list(range(8)), trace=trace)
    _CACHE["last_result"] = res

    out = np.empty((B, C, N), np.float32)
    for core in range(8):
        b, s = core // 4, core % 4
        out[b, :, s * NQ:(s + 1) * NQ] = res.results[core]["o"]
    return out.reshape(B, C, H, W)
